# revision 40
# baseline (speedup 1.0000x reference)
"""Causal self-attention on 8 trn2 NeuronCores.

Sharding: 4 heads x 1 batch per core (core c: batch c//4, heads
4*(c%4)..4*(c%4)+3).  Each core computes Q/K/V projections for its
head group (column-parallel), causal attention, and a row-parallel
partial of the output projection.  Host sums 4 partials per batch and
adds the bias terms (bo, and bv@Wo which is exact because softmax rows
sum to 1).  bk is dropped entirely: its effect on scores is constant
per softmax row, so it cancels exactly.

All matmul operands are bf16 (fp32 PSUM accumulation); softmax
denominator accumulation is fp32 on the vector engine, reduced across
partitions by a single ones-matmul per (head, query-chunk).

Layout (partition dim first):
  x_t  : x chunk transposed -> (model 128-blocks, 512 seq)
  qt/ktt: (head_dim, head, seq)           [proj lhsT = W block]
  vt   : (seq keys, key-tile, head*dh)    [proj lhsT = x block]
  scores: St = (keys, queries) = Kt_blk.T @ Qt_chunk; causal mask is
          PRELOADED into the PSUM bank (DVE) and the diagonal-block
          matmul accumulates onto it with start=False.
  exp(St) (bf16) feeds P@V directly:  attnT = V_blk.T @ exp
  denominator: exp_sum += exp (DVE), then ones.T @ exp_sum
  out-proj: lhsT = a_sb block, rhs = Wo rows slice -> (queries, model)

The kernel body is emitted as a single woven instruction stream
(projection / attention / out-projection generators interleaved by a
Bresenham scheduler) so the in-order PE never waits on the scalar
engine's exp and stays at the 2.4 GHz p-state.
"""

import os

import numpy as np
import ml_dtypes

import concourse.bass as bass
import concourse.bass_isa as bass_isa
import concourse.mybir as mybir
import concourse.tile as tile
from concourse import bacc
from concourse.bass_utils import run_bass_kernel_spmd

F32 = mybir.dt.float32
F32R = mybir.dt.float32r
BF16 = mybir.dt.bfloat16
AF = mybir.ActivationFunctionType
ALU = mybir.AluOpType
BF = ml_dtypes.bfloat16

B = 2
S = 2048
D = 2048
H = 16
DH = 128
NCORES = 8
HPC = 4          # heads per core (single batch per core)
GW = HPC * DH    # 512: projection group width
KT = D // 128    # 16 contraction tiles for projections
NQC = S // 512   # 4 query chunks
NST = S // 128   # 16 key tiles
SCALE = 1.0 / np.sqrt(DH)
NEG = -1e9

_NC_CACHE = {}


def _weave(streams):
    """Interleave unit generators proportionally (Bresenham)."""
    streams = [(it, n) for (it, n) in streams if n > 0]
    total = sum(n for _, n in streams)
    done = [0] * len(streams)
    for t in range(1, total + 1):
        best, bd = -1, -1e9
        for j, (_, n) in enumerate(streams):
            if done[j] >= n:
                continue
            d = n * t / total - done[j]
            if d > bd:
                best, bd = j, d
        next(streams[best][0])
        done[best] += 1
    for it, _ in streams:
        for _ in it:  # drain any miscounted tail
            pass


def _build():
    nc = bacc.Bacc(None, target_bir_lowering=False, debug=False)

    xt = nc.dram_tensor("xt", [KT, 128, S], BF16, kind="ExternalInput")
    wq = nc.dram_tensor("wq", [KT, 128, GW], BF16, kind="ExternalInput")
    wk = nc.dram_tensor("wk", [KT, 128, GW], BF16, kind="ExternalInput")
    wv = nc.dram_tensor("wv", [KT, 128, GW], BF16, kind="ExternalInput")
    wo = nc.dram_tensor("wo", [HPC, 128, D], BF16, kind="ExternalInput")
    bq2 = nc.dram_tensor("bq2", [HPC, 128], F32, kind="ExternalInput")
    mblk = nc.dram_tensor("mblk", [128, 128], F32, kind="ExternalInput")
    onem = nc.dram_tensor("onem", [128, 128], BF16, kind="ExternalInput")
    out = nc.dram_tensor("out", [S, D], BF16, kind="ExternalOutput")

    with tile.TileContext(nc) as tc:
        with (
            tc.tile_pool(name="const", bufs=1) as constp,
            tc.tile_pool(name="xtp", bufs=2) as xtp,
            tc.tile_pool(name="qkv", bufs=1) as qkvp,
            tc.tile_pool(name="expp", bufs=3) as expp,
            tc.tile_pool(name="esp", bufs=2) as esp,
            tc.tile_pool(name="rcp", bufs=2) as rcp,
            tc.tile_pool(name="asb", bufs=12) as asbp,
            tc.tile_pool(name="ostp", bufs=3) as ostp,
            # PSUM bank budget (8): work (qk/v/op) 4 + st 2 + at 2.
            # One FIFO ring for all projection/out-proj tiles: 4-deep
            # pipelining hides every PSUM->SBUF copy without filler.
            tc.tile_pool(name="ps_work", bufs=4, space="PSUM") as ps_work,
            tc.tile_pool(name="ps_st", bufs=2, space="PSUM") as ps_st,
            tc.tile_pool(name="ps_at", bufs=2, space="PSUM") as ps_at,
        ):
            # ---------------- constants (weights) -----------------
            # wq/wk/x0 are 16 per-k tiles so the first LDWEIGHTS/MATMUL
            # waits only on its own 128 KB DMA, not the whole 2 MB
            # tensor: the PE starts ~1 us in instead of ~12 us.
            wq_k = [constp.tile([128, GW], BF16, tag=f"wq{k}", name=f"wq{k}")
                    for k in range(KT)]
            wk_k = [constp.tile([128, GW], BF16, tag=f"wk{k}", name=f"wk{k}")
                    for k in range(KT)]
            x0_k = [constp.tile([128, 512], BF16, tag=f"x0_{k}", name=f"x0_{k}")
                    for k in range(KT)]
            wv_k = [constp.tile([128, GW], BF16, tag=f"wv{k}", name=f"wv{k}")
                    for k in range(KT)]
            wo_t = constp.tile([128, HPC, D], BF16, tag="wo")
            x_tiles = [None] * NQC

            def load_x(c):
                x_t = xtp.tile([128, KT, 512], BF16, tag="x", name=f"x{c}")
                for k in range(KT):
                    nc.sync.dma_start(
                        x_t[:, k, :], xt[k, :, c * 512 : (c + 1) * 512]
                    )
                x_tiles[c] = x_t

            # DMA issue order tracks phase-1 consumption order: tiny
            # constants first (a Q-bias/mask arriving behind megabytes
            # of weights stalls the whole phase-2 pipeline), then
            # {x0,wq} pairs per k (phase-1 Q is k-outer), all-K, all-V,
            # chunk 1, and wo in 128 KB chunks so no single transfer
            # hogs a queue.
            # ---- PE pre-warm -------------------------------------
            # No DMA packet lands before ~9 us (engine/queue boot), so
            # the PE would sit idle and start the real stream at the
            # throttled 1.2 GHz HAM state.  A stream of dummy matmuls
            # on a memset scratch tile keeps the PE busy from ~0.5 us,
            # so HAM is at 2.4 GHz (and stays there) when the first
            # projection matmul fires at ~11 us.
            scr_t = constp.tile([128, 512], BF16, tag="scr")
            nc.gpsimd.memset(scr_t[:], 0.0)
            warm_ps = ps_st.tile([128, 512], F32, tag="st",
                                 name="warm_ps")
            for i in range(10):
                nc.tensor.matmul(
                    warm_ps[:], scr_t[:, 0:128], scr_t[:],
                    start=True, stop=True,
                )

            bq_t = constp.tile([128, HPC], F32, tag="bq")
            mask_t = constp.tile([128, 128], F32, tag="mask")
            ones_m = constp.tile([128, 128], BF16, tag="ones_m")
            x1 = xtp.tile([128, KT, 512], BF16, tag="x", name="x1")
            x_tiles[1] = x1
            # One dma_start lands on ONE of the 16 DMA engines, and a
            # [128, W] tile is 128 row-packets processed serially there
            # (~53 ns each): a whole tile takes ~6.8 us to arrive no
            # matter W, a 64-row half ~3.4 us.  Each queue issues one
            # dma_start per ~650 ns, serially.  The phase-1 tensors are
            # issued in EXACT k-outer consumption order, round-robined
            # over the three DMA-capable queues (sync / scalar /
            # gpsimd), halves first, so delivery tracks consumption
            # with a ~3.5 us lead.
            items = []
            for k in range(8):
                items += [
                    (x0_k[k][0:64, :], xt[k, 0:64, 0:512]),
                    (x0_k[k][64:128, :], xt[k, 64:128, 0:512]),
                    (wq_k[k][0:64, :], wq[k, 0:64, :]),
                    (wq_k[k][64:128, :], wq[k, 64:128, :]),
                ]
            for k in range(8, KT):
                items += [
                    (x0_k[k][0:64, :], xt[k, 0:64, 0:512]),
                    (x0_k[k][64:128, :], xt[k, 64:128, 0:512]),
                    (wq_k[k][:], wq[k]),
                ]
            for k in range(KT):
                items.append((wk_k[k][:], wk[k]))
                if k == 2:
                    items.append((bq_t[:], bq2.rearrange("h p -> p h")))
                    items.append((mask_t[:], mblk[:]))
                    items.append((ones_m[:], onem[:]))
            for k in range(KT):
                items.append((wv_k[k][:], wv[k]))
            queues = [nc.sync, nc.scalar, nc.gpsimd]
            for j, (dst, src) in enumerate(items):
                queues[j % 3].dma_start(dst, src)
            for k in range(KT):
                nc.sync.dma_start(x1[:, k, :], xt[k, :, 512:1024])
            for h in range(HPC):
                for j in range(4):
                    nc.sync.dma_start(
                        wo_t[:, h, j * 512 : (j + 1) * 512],
                        wo[h, :, j * 512 : (j + 1) * 512],
                    )
            # warm the scalar engine's exp table before the main stream
            warm_t = constp.tile([128, 1], F32, tag="warm")
            nc.scalar.activation(warm_t[:], mask_t[:, 0:1], AF.Exp, scale=0.0)

            qt = qkvp.tile([128, HPC, S], BF16, tag="qt")
            ktt = qkvp.tile([128, HPC, S], BF16, tag="ktt")
            vt = qkvp.tile([128, NST, GW], BF16, tag="vt")
            a_sbs = {}  # (h, c) -> normalized attnT tile (bf16)

            # ---------------- stream generators -----------------
            def p0_units():
                """Chunk-0 projections, k-outer: 204 yields.

                Q then K then V, each k-outer with 4 PSUM banks (one
                per head / s-tile), so the PE consumes {x0[k], w[k]}
                pairs in DMA arrival order and compute starts as soon
                as the first 256 KB land instead of after the full
                4 MB.
                """
                load_x(2)
                ps_q = [ps_work.tile([128, 512], F32, tag="ps",
                                     name=f"q0_ps{h}")
                        for h in range(HPC)]
                for k in range(KT):
                    for h in range(HPC):
                        nc.tensor.matmul(
                            ps_q[h][:],
                            wq_k[k][:, h * DH : (h + 1) * DH],
                            x0_k[k][:],
                            start=(k == 0),
                            stop=(k == KT - 1),
                        )
                        if k < KT - 1 or h < HPC - 1:
                            yield
                for h in range(HPC):
                    nc.scalar.add(
                        qt[:, h, 0:512], ps_q[h][:], bq_t[:, h : h + 1]
                    )
                    yield
                ps_k = [ps_work.tile([128, 512], F32, tag="ps",
                                     name=f"k0_ps{h}") for h in range(HPC)]
                for k in range(KT):
                    for h in range(HPC):
                        nc.tensor.matmul(
                            ps_k[h][:],
                            wk_k[k][:, h * DH : (h + 1) * DH],
                            x0_k[k][:],
                            start=(k == 0),
                            stop=(k == KT - 1),
                        )
                        if k < KT - 1 or h < HPC - 1:
                            yield
                for h in range(HPC):
                    nc.vector.tensor_copy(ktt[:, h, 0:512], ps_k[h][:])
                    yield
                ps_v = [ps_work.tile([128, GW], F32, tag="ps",
                                     name=f"v0_ps{s}") for s in range(4)]
                for k in range(KT):
                    for s in range(4):
                        nc.tensor.matmul(
                            ps_v[s][:],
                            x0_k[k][:, s * 128 : (s + 1) * 128],
                            wv_k[k][:],
                            start=(k == 0),
                            stop=(k == KT - 1),
                        )
                        if k < KT - 1 or s < 3:
                            yield
                for s in range(4):
                    nc.vector.tensor_copy(vt[:, s, :], ps_v[s][:])
                    yield

            def p_units(c):
                """Projections for query chunk c>=1: 192 yields.

                All-Q, all-K, all-V order matches the prologue's DMA
                issue order so phase 1 never waits on weights; the
                4-deep work-pool ring hides every PSUM->SBUF copy.
                """
                if c + 2 < NQC:
                    load_x(c + 2)
                x_t = x_tiles[c]

                for w_k, dst, is_q in (
                    (wq_k, qt, True),
                    (wk_k, ktt, False),
                ):
                    for h in range(HPC):
                        ps = ps_work.tile([128, 512], F32, tag="ps",
                                          name="qk_ps")
                        hsl = slice(h * DH, (h + 1) * DH)
                        for k in range(KT):
                            nc.tensor.matmul(
                                ps[:],
                                w_k[k][:, hsl],
                                x_t[:, k, :],
                                start=(k == 0),
                                stop=(k == KT - 1),
                            )
                            if k < KT - 1:
                                yield
                        c0 = c * 512
                        if is_q:
                            nc.scalar.add(
                                dst[:, h, c0 : c0 + 512],
                                ps[:],
                                bq_t[:, h : h + 1],
                            )
                        else:
                            nc.vector.tensor_copy(
                                dst[:, h, c0 : c0 + 512], ps[:]
                            )
                        yield
                for s in range(4):
                    ps = ps_work.tile([128, GW], F32, tag="ps",
                                      name="v_ps")
                    for k in range(KT):
                        nc.tensor.matmul(
                            ps[:],
                            x_t[:, k, s * 128 : (s + 1) * 128],
                            wv_k[k][:],
                            start=(k == 0),
                            stop=(k == KT - 1),
                        )
                        if k < KT - 1:
                            yield
                    nc.vector.tensor_copy(vt[:, c * 4 + s, :], ps[:])
                    yield

            def a_units(c):
                """Attention for query chunk c: 4*(4*(c+1)+5) yields.

                Key tiles are processed diagonal-block-first (i = 4c..
                4c+3 then 0..4c-1) so the start=True PV/score tile is
                the full-width diagonal tile at lo=0.  The causal mask
                is preloaded into the score PSUM bank by the DVE and
                the diagonal matmul accumulates onto it, keeping the
                score->exp->PV chain free of cross-engine hops.
                """
                n_kt = 4 * (c + 1)
                order = list(range(4 * c, n_kt)) + list(range(4 * c))
                qsl = slice(c * 512, (c + 1) * 512)
                for h in range(HPC):
                    hsl = slice(h * DH, (h + 1) * DH)
                    attn_ps = ps_at.tile([128, 512], F32, tag="ps",
                                         name="attn_ps")
                    # bf16 accumulator: each lane only sums 16 exps
                    # (<=0.1% den error), adds run at 2x DVE rate, and
                    # the den matmul consumes it with no cast.
                    exp_sum = esp.tile([128, 512], BF16, tag="es",
                                       name="exp_sum")
                    expts = {}
                    los = {}

                    def score(i):
                        st = ps_st.tile([128, 512], F32, tag="st",
                                        name="st")
                        diag = i >= 4 * c
                        lo = 128 * (i - 4 * c) if diag else 0
                        los[i] = lo
                        nc.tensor.matmul(
                            st[:, lo:],
                            ktt[:, h, i * 128 : (i + 1) * 128],
                            qt[:, h, c * 512 + lo : (c + 1) * 512],
                            start=True,
                            stop=True,
                        )
                        if diag:
                            nc.vector.tensor_tensor(
                                st[:, lo : lo + 128],
                                st[:, lo : lo + 128],
                                mask_t[:],
                                op=ALU.add,
                            )
                        expt = expp.tile([128, 512], BF16, tag="exp",
                                         name="expt")
                        nc.scalar.activation(
                            expt[:, lo:], st[:, lo:], AF.Exp, scale=SCALE
                        )
                        expts[i] = expt

                    def pv_acc(j, first, last):
                        # PV matmul, then the exp_sum accumulate for
                        # the SAME tile.  Issued one unit after
                        # score(i) so the DVE's accumulate never sits
                        # ahead of the next tile's mask preload in the
                        # in-order DVE queue.
                        i = order[j]
                        lo = los[i]
                        nc.tensor.matmul(
                            attn_ps[:, lo:],
                            vt[:, i, hsl],
                            expts[i][:, lo:],
                            start=first,
                            stop=last,
                        )
                        if first:
                            nc.vector.tensor_copy(exp_sum[:], expts[i][:])
                        else:
                            nc.vector.tensor_tensor(
                                exp_sum[:, lo:], exp_sum[:, lo:],
                                expts[i][:, lo:], op=ALU.add,
                            )

                    score(order[0])
                    yield
                    for j in range(1, n_kt):
                        score(order[j])
                        pv_acc(j - 1, j == 1, False)
                        yield
                    pv_acc(n_kt - 1, n_kt == 1, True)
                    yield
                    yield  # spacer: filler covers the DVE accum tail
                    yield
                    den_ps = ps_st.tile([128, 512], F32, tag="st",
                                        name="den_ps")
                    nc.tensor.matmul(
                        den_ps[:], ones_m[:], exp_sum[:],
                        start=True, stop=True,
                    )
                    yield
                    yield  # spacer: filler covers the recip/mult chain
                    rc = rcp.tile([128, 512], F32, tag="rc", name="rc")
                    nc.vector.reciprocal_approx_fast(
                        out=rc[:], in_=den_ps[:]
                    )
                    a_sb = asbp.tile([128, 512], BF16, tag="attnT",
                                     name="a_sb")
                    nc.vector.tensor_tensor(
                        a_sb[:], attn_ps[:], rc[:], op=ALU.mult
                    )
                    a_sbs[(h, c)] = a_sb
                    yield

            def op_units(c):
                """Out-projection for query chunk c: 64 yields.

                The four nch tiles of one 128-query row block land in a
                single [128, 2048] SBUF tile and leave as ONE contiguous
                512 KB DMA: per-call issue cost (~650 ns of serial
                DMA_DIRECT2D) is paid 16x per core instead of 128x, so
                the post-compute drain tail collapses.
                """
                for qs in range(4):
                    row0 = c * 512 + qs * 128
                    o_row = ostp.tile([128, D], BF16, tag="ost",
                                      name="o_row")
                    for nch in range(4):
                        ps = ps_work.tile([128, 512], F32, tag="ps",
                                        name="op_ps")
                        for h in range(HPC):
                            nc.tensor.matmul(
                                ps[:],
                                a_sbs[(h, c)][:, qs * 128 : (qs + 1) * 128],
                                wo_t[:, h, nch * 512 : (nch + 1) * 512],
                                start=(h == 0),
                                stop=(h == HPC - 1),
                            )
                            if h < HPC - 1:
                                yield
                        nc.vector.tensor_copy(
                            o_row[:, nch * 512 : nch * 512 + 384],
                            ps[:, :384],
                        )
                        nc.scalar.copy(
                            o_row[:, nch * 512 + 384 : (nch + 1) * 512],
                            ps[:, 384:],
                        )
                        yield
                    if c == 3 and qs == 3:
                        # final transfer: two partition-halves on two
                        # queues halve the post-compute drain
                        nc.sync.dma_start(
                            out[row0 : row0 + 64, :], o_row[0:64, :]
                        )
                        nc.gpsimd.dma_start(
                            out[row0 + 64 : row0 + 128, :],
                            o_row[64:128, :],
                        )
                    else:
                        nc.sync.dma_start(
                            out[row0 : row0 + 128, :], o_row[:]
                        )

            # ---------------- phase schedule -----------------
            NA = lambda c: 4 * (4 * (c + 1) + 6)
            _weave([(p0_units(), 201)])
            _weave([(p_units(1), 192), (a_units(0), NA(0))])
            _weave([(p_units(2), 192), (a_units(1), NA(1)),
                    (op_units(0), 64)])
            _weave([(p_units(3), 192), (a_units(2), NA(2))])
            _weave([(a_units(3), NA(3)), (op_units(1), 64),
                    (op_units(2), 64)])
            _weave([(op_units(3), 64)])
    nc.compile()
    return nc


def _get_nc():
    if "nc" not in _NC_CACHE:
        _NC_CACHE["nc"] = _build()
    return _NC_CACHE["nc"]


def kernel(x, mask, Wq, bq, Wk, bk, Wv, bv, Wo, bo):
    x = np.asarray(x, dtype=np.float32)
    Wq = np.asarray(Wq, dtype=np.float32)
    Wk = np.asarray(Wk, dtype=np.float32)
    Wv = np.asarray(Wv, dtype=np.float32)
    Wo = np.asarray(Wo, dtype=np.float32)
    bq = np.asarray(bq, dtype=np.float32)
    bv = np.asarray(bv, dtype=np.float32)
    bo = np.asarray(bo, dtype=np.float32)

    nc = _get_nc()

    # per-batch transposed x, bf16: (KT, 128, S)
    xts = [
        np.ascontiguousarray(
            x[b].T.reshape(KT, 128, S)
        ).astype(BF)
        for b in range(B)
    ]
    kl = np.arange(128)
    mblk = np.where(kl[:, None] <= kl[None, :], 0.0, NEG).astype(np.float32)
    onem = np.ones((128, 128), dtype=BF)

    in_maps = []
    for c in range(NCORES):
        b, g = c // HPC, c % HPC
        cols = slice(g * GW, (g + 1) * GW)
        in_maps.append(
            {
                "xt": xts[b],
                "wq": np.ascontiguousarray(Wq[:, cols]).reshape(
                    KT, 128, GW
                ).astype(BF),
                "wk": np.ascontiguousarray(Wk[:, cols]).reshape(
                    KT, 128, GW
                ).astype(BF),
                "wv": np.ascontiguousarray(Wv[:, cols]).reshape(
                    KT, 128, GW
                ).astype(BF),
                "wo": np.ascontiguousarray(Wo[cols, :]).reshape(
                    HPC, 128, D
                ).astype(BF),
                "bq2": np.ascontiguousarray(bq[cols]).reshape(HPC, 128),
                "mblk": mblk,
                "onem": onem,
            }
        )

    trace = bool(int(os.environ.get("BASS_ATTN_TRACE", "0")))
    try:
        res = run_bass_kernel_spmd(
            nc, in_maps, core_ids=list(range(NCORES)), trace=trace
        )
    except Exception:
        # transient device errors (e.g. a wedged core from a prior run)
        # usually clear on retry
        res = run_bass_kernel_spmd(
            nc, in_maps, core_ids=list(range(NCORES)), trace=trace
        )
    if trace:
        _NC_CACHE["last_result"] = res

    outs = np.empty((B, S, D), dtype=np.float32)
    for b in range(B):
        acc = res.results[b * HPC]["out"].astype(np.float32)
        for g in range(1, HPC):
            acc += res.results[b * HPC + g]["out"].astype(np.float32)
        outs[b] = acc
    # bv's effect: softmax rows sum to 1, so attn = attn_nobv + bv per head
    # -> out += bv @ Wo (exact). bo added directly. bk cancels in softmax.
    corr = (bv.astype(np.float64) @ Wo.astype(np.float64)) + np.asarray(
        bo, dtype=np.float64
    )
    outs += corr.astype(np.float32)
    return outs



# revision 42
# speedup vs baseline: 1.0278x; 1.0278x over previous
"""Causal self-attention on 8 trn2 NeuronCores.

Sharding: 4 heads x 1 batch per core (core c: batch c//4, heads
4*(c%4)..4*(c%4)+3).  Each core computes Q/K/V projections for its
head group (column-parallel), causal attention, and a row-parallel
partial of the output projection.  Host sums 4 partials per batch and
adds the bias terms (bo, and bv@Wo which is exact because softmax rows
sum to 1).  bk is dropped entirely: its effect on scores is constant
per softmax row, so it cancels exactly.

All matmul operands are bf16 (fp32 PSUM accumulation); softmax
denominator accumulation is fp32 on the vector engine, reduced across
partitions by a single ones-matmul per (head, query-chunk).

Layout (partition dim first):
  x_t  : x chunk transposed -> (model 128-blocks, 512 seq)
  qt/ktt: (head_dim, head, seq)           [proj lhsT = W block]
  vt   : (seq keys, key-tile, head*dh)    [proj lhsT = x block]
  scores: St = (keys, queries) = Kt_blk.T @ Qt_chunk; causal mask is
          PRELOADED into the PSUM bank (DVE) and the diagonal-block
          matmul accumulates onto it with start=False.
  exp(St) (bf16) feeds P@V directly:  attnT = V_blk.T @ exp
  denominator: exp_sum += exp (DVE), then ones.T @ exp_sum
  out-proj: lhsT = a_sb block, rhs = Wo rows slice -> (queries, model)

The kernel body is emitted as a single woven instruction stream
(projection / attention / out-projection generators interleaved by a
Bresenham scheduler) so the in-order PE never waits on the scalar
engine's exp and stays at the 2.4 GHz p-state.
"""

import os

import numpy as np
import ml_dtypes

import concourse.bass as bass
import concourse.bass_isa as bass_isa
import concourse.mybir as mybir
import concourse.tile as tile
from concourse import bacc
from concourse.bass_utils import run_bass_kernel_spmd

F32 = mybir.dt.float32
F32R = mybir.dt.float32r
BF16 = mybir.dt.bfloat16
AF = mybir.ActivationFunctionType
ALU = mybir.AluOpType
BF = ml_dtypes.bfloat16

B = 2
S = 2048
D = 2048
H = 16
DH = 128
NCORES = 8
HPC = 4          # heads per core (single batch per core)
GW = HPC * DH    # 512: projection group width
KT = D // 128    # 16 contraction tiles for projections
NQC = S // 512   # 4 query chunks
NST = S // 128   # 16 key tiles
SCALE = 1.0 / np.sqrt(DH)
NEG = -1e9

_NC_CACHE = {}


def _weave(streams):
    """Interleave unit generators proportionally (Bresenham)."""
    streams = [(it, n) for (it, n) in streams if n > 0]
    total = sum(n for _, n in streams)
    done = [0] * len(streams)
    for t in range(1, total + 1):
        best, bd = -1, -1e9
        for j, (_, n) in enumerate(streams):
            if done[j] >= n:
                continue
            d = n * t / total - done[j]
            if d > bd:
                best, bd = j, d
        next(streams[best][0])
        done[best] += 1
    for it, _ in streams:
        for _ in it:  # drain any miscounted tail
            pass


def _build():
    nc = bacc.Bacc(None, target_bir_lowering=False, debug=False)

    xt = nc.dram_tensor("xt", [KT, 128, S], BF16, kind="ExternalInput")
    wq = nc.dram_tensor("wq", [KT, 128, GW], BF16, kind="ExternalInput")
    wk = nc.dram_tensor("wk", [KT, 128, GW], BF16, kind="ExternalInput")
    wv = nc.dram_tensor("wv", [KT, 128, GW], BF16, kind="ExternalInput")
    wo = nc.dram_tensor("wo", [HPC, 128, D], BF16, kind="ExternalInput")
    bq2 = nc.dram_tensor("bq2", [HPC, 128], F32, kind="ExternalInput")
    mblk = nc.dram_tensor("mblk", [128, 128], F32, kind="ExternalInput")
    onem = nc.dram_tensor("onem", [128, 128], BF16, kind="ExternalInput")
    out = nc.dram_tensor("out", [S, D], BF16, kind="ExternalOutput")

    with tile.TileContext(nc) as tc:
        with (
            tc.tile_pool(name="const", bufs=1) as constp,
            tc.tile_pool(name="xtp", bufs=2) as xtp,
            tc.tile_pool(name="qkv", bufs=1) as qkvp,
            tc.tile_pool(name="expp", bufs=3) as expp,
            tc.tile_pool(name="esp", bufs=2) as esp,
            tc.tile_pool(name="rcp", bufs=2) as rcp,
            tc.tile_pool(name="asb", bufs=12) as asbp,
            tc.tile_pool(name="ostp", bufs=3) as ostp,
            # PSUM bank budget (8): work (qk/v/op) 4 + st 2 + at 2.
            # One FIFO ring for all projection/out-proj tiles: 4-deep
            # pipelining hides every PSUM->SBUF copy without filler.
            tc.tile_pool(name="ps_work", bufs=4, space="PSUM") as ps_work,
            tc.tile_pool(name="ps_st", bufs=2, space="PSUM") as ps_st,
            tc.tile_pool(name="ps_at", bufs=2, space="PSUM") as ps_at,
        ):
            # ---------------- constants (weights) -----------------
            # wq/wk/x0 are 16 per-k tiles so the first LDWEIGHTS/MATMUL
            # waits only on its own 128 KB DMA, not the whole 2 MB
            # tensor: the PE starts ~1 us in instead of ~12 us.
            wq_k = [constp.tile([128, GW], BF16, tag=f"wq{k}", name=f"wq{k}")
                    for k in range(KT)]
            wk_k = [constp.tile([128, GW], BF16, tag=f"wk{k}", name=f"wk{k}")
                    for k in range(KT)]
            x0_k = [constp.tile([128, 512], BF16, tag=f"x0_{k}", name=f"x0_{k}")
                    for k in range(KT)]
            wv_k = [constp.tile([128, GW], BF16, tag=f"wv{k}", name=f"wv{k}")
                    for k in range(KT)]
            wo_t = constp.tile([128, HPC, D], BF16, tag="wo")
            x_tiles = [None] * NQC

            def load_x(c):
                x_t = xtp.tile([128, KT, 512], BF16, tag="x", name=f"x{c}")
                for k in range(KT):
                    nc.sync.dma_start(
                        x_t[:, k, :], xt[k, :, c * 512 : (c + 1) * 512]
                    )
                x_tiles[c] = x_t

            # DMA issue order tracks phase-1 consumption order: tiny
            # constants first (a Q-bias/mask arriving behind megabytes
            # of weights stalls the whole phase-2 pipeline), then
            # {x0,wq} pairs per k (phase-1 Q is k-outer), all-K, all-V,
            # chunk 1, and wo in 128 KB chunks so no single transfer
            # hogs a queue.
            # ---- PE pre-warm -------------------------------------
            # No DMA packet lands before ~9 us (engine/queue boot), so
            # the PE would sit idle and start the real stream at the
            # throttled 1.2 GHz HAM state.  A stream of dummy matmuls
            # on a memset scratch tile keeps the PE busy from ~0.5 us,
            # so HAM is at 2.4 GHz (and stays there) when the first
            # projection matmul fires at ~11 us.
            scr_t = constp.tile([128, 512], BF16, tag="scr")
            nc.gpsimd.memset(scr_t[:], 0.0)
            warm_ps = ps_st.tile([128, 512], F32, tag="st",
                                 name="warm_ps")
            for i in range(10):
                nc.tensor.matmul(
                    warm_ps[:], scr_t[:, 0:128], scr_t[:],
                    start=True, stop=True,
                )

            bq_t = constp.tile([128, HPC], F32, tag="bq")
            mask_t = constp.tile([128, 128], F32, tag="mask")
            ones_m = constp.tile([128, 128], BF16, tag="ones_m")
            x1 = xtp.tile([128, KT, 512], BF16, tag="x", name="x1")
            x_tiles[1] = x1
            # One dma_start lands on ONE of the 16 DMA engines, and a
            # [128, W] tile is 128 row-packets processed serially there
            # (~53 ns each): a whole tile takes ~6.8 us to arrive no
            # matter W, a 64-row half ~3.4 us.  Each queue issues one
            # dma_start per ~650 ns, serially.  The phase-1 tensors are
            # issued in EXACT k-outer consumption order, round-robined
            # over the three DMA-capable queues (sync / scalar /
            # gpsimd), halves first, so delivery tracks consumption
            # with a ~3.5 us lead.
            items = []
            for k in range(8):
                items += [
                    (x0_k[k][0:64, :], xt[k, 0:64, 0:512]),
                    (x0_k[k][64:128, :], xt[k, 64:128, 0:512]),
                    (wq_k[k][0:64, :], wq[k, 0:64, :]),
                    (wq_k[k][64:128, :], wq[k, 64:128, :]),
                ]
            for k in range(8, KT):
                items += [
                    (x0_k[k][0:64, :], xt[k, 0:64, 0:512]),
                    (x0_k[k][64:128, :], xt[k, 64:128, 0:512]),
                    (wq_k[k][:], wq[k]),
                ]
            for k in range(KT):
                items.append((wk_k[k][:], wk[k]))
                if k == 2:
                    items.append((bq_t[:], bq2.rearrange("h p -> p h")))
                    items.append((mask_t[:], mblk[:]))
                    items.append((ones_m[:], onem[:]))
            for k in range(KT):
                items.append((wv_k[k][:], wv[k]))
            queues = [nc.sync, nc.scalar, nc.gpsimd]
            for j, (dst, src) in enumerate(items):
                queues[j % 3].dma_start(dst, src)
            for k in range(KT):
                nc.sync.dma_start(x1[:, k, :], xt[k, :, 512:1024])
            for h in range(HPC):
                for j in range(4):
                    nc.sync.dma_start(
                        wo_t[:, h, j * 512 : (j + 1) * 512],
                        wo[h, :, j * 512 : (j + 1) * 512],
                    )
            # warm the scalar engine's exp table before the main stream
            warm_t = constp.tile([128, 1], F32, tag="warm")
            nc.scalar.activation(warm_t[:], mask_t[:, 0:1], AF.Exp, scale=0.0)

            qt = qkvp.tile([128, HPC, S], BF16, tag="qt")
            ktt = qkvp.tile([128, HPC, S], BF16, tag="ktt")
            vt = qkvp.tile([128, NST, GW], BF16, tag="vt")
            a_sbs = {}  # (h, c) -> normalized attnT tile (bf16)

            # ---------------- stream generators -----------------
            def p0_units():
                """Chunk-0 projections, k-outer: 204 yields.

                Q then K then V, each k-outer with 4 PSUM banks (one
                per head / s-tile), so the PE consumes {x0[k], w[k]}
                pairs in DMA arrival order and compute starts as soon
                as the first 256 KB land instead of after the full
                4 MB.
                """
                load_x(2)
                ps_q = [ps_work.tile([128, 512], F32, tag="ps",
                                     name=f"q0_ps{h}")
                        for h in range(HPC)]
                for k in range(KT):
                    for h in range(HPC):
                        nc.tensor.matmul(
                            ps_q[h][:],
                            wq_k[k][:, h * DH : (h + 1) * DH],
                            x0_k[k][:],
                            start=(k == 0),
                            stop=(k == KT - 1),
                        )
                        if k < KT - 1 or h < HPC - 1:
                            yield
                for h in range(HPC):
                    # DVE, not ACT: the scalar queue is busy issuing
                    # prologue DMAs and would stall the PSUM ring
                    nc.vector.tensor_scalar_add(
                        qt[:, h, 0:512], ps_q[h][:], bq_t[:, h : h + 1]
                    )
                    yield
                ps_k = [ps_work.tile([128, 512], F32, tag="ps",
                                     name=f"k0_ps{h}") for h in range(HPC)]
                for k in range(KT):
                    for h in range(HPC):
                        nc.tensor.matmul(
                            ps_k[h][:],
                            wk_k[k][:, h * DH : (h + 1) * DH],
                            x0_k[k][:],
                            start=(k == 0),
                            stop=(k == KT - 1),
                        )
                        if k < KT - 1 or h < HPC - 1:
                            yield
                for h in range(HPC):
                    nc.vector.tensor_copy(ktt[:, h, 0:512], ps_k[h][:])
                    yield
                ps_v = [ps_work.tile([128, GW], F32, tag="ps",
                                     name=f"v0_ps{s}") for s in range(4)]
                for k in range(KT):
                    for s in range(4):
                        nc.tensor.matmul(
                            ps_v[s][:],
                            x0_k[k][:, s * 128 : (s + 1) * 128],
                            wv_k[k][:],
                            start=(k == 0),
                            stop=(k == KT - 1),
                        )
                        if k < KT - 1 or s < 3:
                            yield
                for s in range(4):
                    nc.vector.tensor_copy(vt[:, s, :], ps_v[s][:])
                    yield

            def p_units(c):
                """Projections for query chunk c>=1: 192 yields.

                All-Q, all-K, all-V order matches the prologue's DMA
                issue order so phase 1 never waits on weights; the
                4-deep work-pool ring hides every PSUM->SBUF copy.
                """
                if c + 2 < NQC:
                    load_x(c + 2)
                x_t = x_tiles[c]

                for w_k, dst, is_q in (
                    (wq_k, qt, True),
                    (wk_k, ktt, False),
                ):
                    for h in range(HPC):
                        ps = ps_work.tile([128, 512], F32, tag="ps",
                                          name="qk_ps")
                        hsl = slice(h * DH, (h + 1) * DH)
                        for k in range(KT):
                            nc.tensor.matmul(
                                ps[:],
                                w_k[k][:, hsl],
                                x_t[:, k, :],
                                start=(k == 0),
                                stop=(k == KT - 1),
                            )
                            if k < KT - 1:
                                yield
                        c0 = c * 512
                        if is_q:
                            nc.scalar.add(
                                dst[:, h, c0 : c0 + 512],
                                ps[:],
                                bq_t[:, h : h + 1],
                            )
                        else:
                            nc.vector.tensor_copy(
                                dst[:, h, c0 : c0 + 512], ps[:]
                            )
                        yield
                for s in range(4):
                    ps = ps_work.tile([128, GW], F32, tag="ps",
                                      name="v_ps")
                    for k in range(KT):
                        nc.tensor.matmul(
                            ps[:],
                            x_t[:, k, s * 128 : (s + 1) * 128],
                            wv_k[k][:],
                            start=(k == 0),
                            stop=(k == KT - 1),
                        )
                        if k < KT - 1:
                            yield
                    nc.vector.tensor_copy(vt[:, c * 4 + s, :], ps[:])
                    yield

            def a_units(c):
                """Attention for query chunk c: 4*(4*(c+1)+5) yields.

                Key tiles are processed diagonal-block-first (i = 4c..
                4c+3 then 0..4c-1) so the start=True PV/score tile is
                the full-width diagonal tile at lo=0.  The causal mask
                is preloaded into the score PSUM bank by the DVE and
                the diagonal matmul accumulates onto it, keeping the
                score->exp->PV chain free of cross-engine hops.
                """
                n_kt = 4 * (c + 1)
                order = list(range(4 * c, n_kt)) + list(range(4 * c))
                qsl = slice(c * 512, (c + 1) * 512)
                for h in range(HPC):
                    hsl = slice(h * DH, (h + 1) * DH)
                    attn_ps = ps_at.tile([128, 512], F32, tag="ps",
                                         name="attn_ps")
                    # bf16 accumulator: each lane only sums 16 exps
                    # (<=0.1% den error), adds run at 2x DVE rate, and
                    # the den matmul consumes it with no cast.
                    exp_sum = esp.tile([128, 512], BF16, tag="es",
                                       name="exp_sum")
                    expts = {}
                    los = {}

                    def score(i):
                        st = ps_st.tile([128, 512], F32, tag="st",
                                        name="st")
                        diag = i >= 4 * c
                        lo = 128 * (i - 4 * c) if diag else 0
                        los[i] = lo
                        nc.tensor.matmul(
                            st[:, lo:],
                            ktt[:, h, i * 128 : (i + 1) * 128],
                            qt[:, h, c * 512 + lo : (c + 1) * 512],
                            start=True,
                            stop=True,
                        )
                        if diag:
                            nc.vector.tensor_tensor(
                                st[:, lo : lo + 128],
                                st[:, lo : lo + 128],
                                mask_t[:],
                                op=ALU.add,
                            )
                        expt = expp.tile([128, 512], BF16, tag="exp",
                                         name="expt")
                        nc.scalar.activation(
                            expt[:, lo:], st[:, lo:], AF.Exp, scale=SCALE
                        )
                        expts[i] = expt

                    def pv_acc(j, first, last):
                        # PV matmul, then the exp_sum accumulate for
                        # the SAME tile.  Issued one unit after
                        # score(i) so the DVE's accumulate never sits
                        # ahead of the next tile's mask preload in the
                        # in-order DVE queue.
                        i = order[j]
                        lo = los[i]
                        nc.tensor.matmul(
                            attn_ps[:, lo:],
                            vt[:, i, hsl],
                            expts[i][:, lo:],
                            start=first,
                            stop=last,
                        )
                        if first:
                            nc.vector.tensor_copy(exp_sum[:], expts[i][:])
                        else:
                            nc.vector.tensor_tensor(
                                exp_sum[:, lo:], exp_sum[:, lo:],
                                expts[i][:, lo:], op=ALU.add,
                            )

                    score(order[0])
                    yield
                    for j in range(1, n_kt):
                        score(order[j])
                        pv_acc(j - 1, j == 1, False)
                        yield
                    pv_acc(n_kt - 1, n_kt == 1, True)
                    yield
                    yield  # spacer: filler covers the DVE accum tail
                    yield
                    den_ps = ps_st.tile([128, 512], F32, tag="st",
                                        name="den_ps")
                    nc.tensor.matmul(
                        den_ps[:], ones_m[:], exp_sum[:],
                        start=True, stop=True,
                    )
                    yield
                    yield  # spacer: filler covers the recip/mult chain
                    rc = rcp.tile([128, 512], F32, tag="rc", name="rc")
                    nc.vector.reciprocal_approx_fast(
                        out=rc[:], in_=den_ps[:]
                    )
                    a_sb = asbp.tile([128, 512], BF16, tag="attnT",
                                     name="a_sb")
                    nc.vector.tensor_tensor(
                        a_sb[:], attn_ps[:], rc[:], op=ALU.mult
                    )
                    a_sbs[(h, c)] = a_sb
                    yield

            def op_units(c):
                """Out-projection for query chunk c: 64 yields.

                The four nch tiles of one 128-query row block land in a
                single [128, 2048] SBUF tile and leave as ONE contiguous
                512 KB DMA: per-call issue cost (~650 ns of serial
                DMA_DIRECT2D) is paid 16x per core instead of 128x, so
                the post-compute drain tail collapses.
                """
                for qs in range(4):
                    row0 = c * 512 + qs * 128
                    o_row = ostp.tile([128, D], BF16, tag="ost",
                                      name="o_row")
                    for nch in range(4):
                        ps = ps_work.tile([128, 512], F32, tag="ps",
                                        name="op_ps")
                        for h in range(HPC):
                            nc.tensor.matmul(
                                ps[:],
                                a_sbs[(h, c)][:, qs * 128 : (qs + 1) * 128],
                                wo_t[:, h, nch * 512 : (nch + 1) * 512],
                                start=(h == 0),
                                stop=(h == HPC - 1),
                            )
                            if h < HPC - 1:
                                yield
                        nc.vector.tensor_copy(
                            o_row[:, nch * 512 : nch * 512 + 384],
                            ps[:, :384],
                        )
                        nc.scalar.copy(
                            o_row[:, nch * 512 + 384 : (nch + 1) * 512],
                            ps[:, 384:],
                        )
                        yield
                    if c == 3 and qs == 3:
                        # final transfer: two partition-halves on two
                        # queues halve the post-compute drain
                        nc.sync.dma_start(
                            out[row0 : row0 + 64, :], o_row[0:64, :]
                        )
                        nc.gpsimd.dma_start(
                            out[row0 + 64 : row0 + 128, :],
                            o_row[64:128, :],
                        )
                    else:
                        nc.sync.dma_start(
                            out[row0 : row0 + 128, :], o_row[:]
                        )

            # ---------------- phase schedule -----------------
            NA = lambda c: 4 * (4 * (c + 1) + 6)
            _weave([(p0_units(), 201)])
            _weave([(p_units(1), 192), (a_units(0), NA(0))])
            _weave([(p_units(2), 192), (a_units(1), NA(1)),
                    (op_units(0), 64)])
            _weave([(p_units(3), 192), (a_units(2), NA(2))])
            _weave([(a_units(3), NA(3)), (op_units(1), 64),
                    (op_units(2), 64)])
            _weave([(op_units(3), 64)])
    nc.compile()
    return nc


def _get_nc():
    if "nc" not in _NC_CACHE:
        _NC_CACHE["nc"] = _build()
    return _NC_CACHE["nc"]


def kernel(x, mask, Wq, bq, Wk, bk, Wv, bv, Wo, bo):
    x = np.asarray(x, dtype=np.float32)
    Wq = np.asarray(Wq, dtype=np.float32)
    Wk = np.asarray(Wk, dtype=np.float32)
    Wv = np.asarray(Wv, dtype=np.float32)
    Wo = np.asarray(Wo, dtype=np.float32)
    bq = np.asarray(bq, dtype=np.float32)
    bv = np.asarray(bv, dtype=np.float32)
    bo = np.asarray(bo, dtype=np.float32)

    nc = _get_nc()

    # per-batch transposed x, bf16: (KT, 128, S)
    xts = [
        np.ascontiguousarray(
            x[b].T.reshape(KT, 128, S)
        ).astype(BF)
        for b in range(B)
    ]
    kl = np.arange(128)
    mblk = np.where(kl[:, None] <= kl[None, :], 0.0, NEG).astype(np.float32)
    onem = np.ones((128, 128), dtype=BF)

    in_maps = []
    for c in range(NCORES):
        b, g = c // HPC, c % HPC
        cols = slice(g * GW, (g + 1) * GW)
        in_maps.append(
            {
                "xt": xts[b],
                "wq": np.ascontiguousarray(Wq[:, cols]).reshape(
                    KT, 128, GW
                ).astype(BF),
                "wk": np.ascontiguousarray(Wk[:, cols]).reshape(
                    KT, 128, GW
                ).astype(BF),
                "wv": np.ascontiguousarray(Wv[:, cols]).reshape(
                    KT, 128, GW
                ).astype(BF),
                "wo": np.ascontiguousarray(Wo[cols, :]).reshape(
                    HPC, 128, D
                ).astype(BF),
                "bq2": np.ascontiguousarray(bq[cols]).reshape(HPC, 128),
                "mblk": mblk,
                "onem": onem,
            }
        )

    trace = bool(int(os.environ.get("BASS_ATTN_TRACE", "0")))
    try:
        res = run_bass_kernel_spmd(
            nc, in_maps, core_ids=list(range(NCORES)), trace=trace
        )
    except Exception:
        # transient device errors (e.g. a wedged core from a prior run)
        # usually clear on retry
        res = run_bass_kernel_spmd(
            nc, in_maps, core_ids=list(range(NCORES)), trace=trace
        )
    if trace:
        _NC_CACHE["last_result"] = res

    outs = np.empty((B, S, D), dtype=np.float32)
    for b in range(B):
        acc = res.results[b * HPC]["out"].astype(np.float32)
        for g in range(1, HPC):
            acc += res.results[b * HPC + g]["out"].astype(np.float32)
        outs[b] = acc
    # bv's effect: softmax rows sum to 1, so attn = attn_nobv + bv per head
    # -> out += bv @ Wo (exact). bo added directly. bk cancels in softmax.
    corr = (bv.astype(np.float64) @ Wo.astype(np.float64)) + np.asarray(
        bo, dtype=np.float64
    )
    outs += corr.astype(np.float32)
    return outs



# revision 43
# speedup vs baseline: 1.0331x; 1.0052x over previous
"""Causal self-attention on 8 trn2 NeuronCores.

Sharding: 4 heads x 1 batch per core (core c: batch c//4, heads
4*(c%4)..4*(c%4)+3).  Each core computes Q/K/V projections for its
head group (column-parallel), causal attention, and a row-parallel
partial of the output projection.  Host sums 4 partials per batch and
adds the bias terms (bo, and bv@Wo which is exact because softmax rows
sum to 1).  bk is dropped entirely: its effect on scores is constant
per softmax row, so it cancels exactly.

All matmul operands are bf16 (fp32 PSUM accumulation); softmax
denominator accumulation is fp32 on the vector engine, reduced across
partitions by a single ones-matmul per (head, query-chunk).

Layout (partition dim first):
  x_t  : x chunk transposed -> (model 128-blocks, 512 seq)
  qt/ktt: (head_dim, head, seq)           [proj lhsT = W block]
  vt   : (seq keys, key-tile, head*dh)    [proj lhsT = x block]
  scores: St = (keys, queries) = Kt_blk.T @ Qt_chunk; causal mask is
          PRELOADED into the PSUM bank (DVE) and the diagonal-block
          matmul accumulates onto it with start=False.
  exp(St) (bf16) feeds P@V directly:  attnT = V_blk.T @ exp
  denominator: exp_sum += exp (DVE), then ones.T @ exp_sum
  out-proj: lhsT = a_sb block, rhs = Wo rows slice -> (queries, model)

The kernel body is emitted as a single woven instruction stream
(projection / attention / out-projection generators interleaved by a
Bresenham scheduler) so the in-order PE never waits on the scalar
engine's exp and stays at the 2.4 GHz p-state.
"""

import os

import numpy as np
import ml_dtypes

import concourse.bass as bass
import concourse.bass_isa as bass_isa
import concourse.mybir as mybir
import concourse.tile as tile
from concourse import bacc
from concourse.bass_utils import run_bass_kernel_spmd

F32 = mybir.dt.float32
F32R = mybir.dt.float32r
BF16 = mybir.dt.bfloat16
AF = mybir.ActivationFunctionType
ALU = mybir.AluOpType
BF = ml_dtypes.bfloat16

B = 2
S = 2048
D = 2048
H = 16
DH = 128
NCORES = 8
HPC = 4          # heads per core (single batch per core)
GW = HPC * DH    # 512: projection group width
KT = D // 128    # 16 contraction tiles for projections
NQC = S // 512   # 4 query chunks
NST = S // 128   # 16 key tiles
SCALE = 1.0 / np.sqrt(DH)
NEG = -1e9

_NC_CACHE = {}


def _weave(streams):
    """Interleave unit generators proportionally (Bresenham)."""
    streams = [(it, n) for (it, n) in streams if n > 0]
    total = sum(n for _, n in streams)
    done = [0] * len(streams)
    for t in range(1, total + 1):
        best, bd = -1, -1e9
        for j, (_, n) in enumerate(streams):
            if done[j] >= n:
                continue
            d = n * t / total - done[j]
            if d > bd:
                best, bd = j, d
        next(streams[best][0])
        done[best] += 1
    for it, _ in streams:
        for _ in it:  # drain any miscounted tail
            pass


def _build():
    nc = bacc.Bacc(None, target_bir_lowering=False, debug=False)

    xt = nc.dram_tensor("xt", [KT, 128, S], BF16, kind="ExternalInput")
    wq = nc.dram_tensor("wq", [KT, 128, GW], BF16, kind="ExternalInput")
    wk = nc.dram_tensor("wk", [KT, 128, GW], BF16, kind="ExternalInput")
    wv = nc.dram_tensor("wv", [KT, 128, GW], BF16, kind="ExternalInput")
    wo = nc.dram_tensor("wo", [HPC, 128, D], BF16, kind="ExternalInput")
    bq2 = nc.dram_tensor("bq2", [HPC, 128], F32, kind="ExternalInput")
    mblk = nc.dram_tensor("mblk", [128, 128], F32, kind="ExternalInput")
    onem = nc.dram_tensor("onem", [128, 128], BF16, kind="ExternalInput")
    out = nc.dram_tensor("out", [S, D], BF16, kind="ExternalOutput")

    with tile.TileContext(nc) as tc:
        with (
            tc.tile_pool(name="const", bufs=1) as constp,
            tc.tile_pool(name="xtp", bufs=2) as xtp,
            tc.tile_pool(name="qkv", bufs=1) as qkvp,
            tc.tile_pool(name="expp", bufs=3) as expp,
            tc.tile_pool(name="esp", bufs=2) as esp,
            tc.tile_pool(name="rcp", bufs=2) as rcp,
            tc.tile_pool(name="asb", bufs=12) as asbp,
            tc.tile_pool(name="ostp", bufs=3) as ostp,
            # PSUM bank budget (8): work (qk/v/op) 4 + st 2 + at 2.
            # One FIFO ring for all projection/out-proj tiles: 4-deep
            # pipelining hides every PSUM->SBUF copy without filler.
            tc.tile_pool(name="ps_work", bufs=4, space="PSUM") as ps_work,
            tc.tile_pool(name="ps_st", bufs=2, space="PSUM") as ps_st,
            tc.tile_pool(name="ps_at", bufs=2, space="PSUM") as ps_at,
        ):
            # ---------------- constants (weights) -----------------
            # wq/wk/x0 are 16 per-k tiles so the first LDWEIGHTS/MATMUL
            # waits only on its own 128 KB DMA, not the whole 2 MB
            # tensor: the PE starts ~1 us in instead of ~12 us.
            wq_k = [constp.tile([128, GW], BF16, tag=f"wq{k}", name=f"wq{k}")
                    for k in range(KT)]
            wk_k = [constp.tile([128, GW], BF16, tag=f"wk{k}", name=f"wk{k}")
                    for k in range(KT)]
            x0_k = [constp.tile([128, 512], BF16, tag=f"x0_{k}", name=f"x0_{k}")
                    for k in range(KT)]
            wv_k = [constp.tile([128, GW], BF16, tag=f"wv{k}", name=f"wv{k}")
                    for k in range(KT)]
            wo_t = constp.tile([128, HPC, D], BF16, tag="wo")
            x_tiles = [None] * NQC

            def load_x(c):
                x_t = xtp.tile([128, KT, 512], BF16, tag="x", name=f"x{c}")
                for k in range(KT):
                    nc.sync.dma_start(
                        x_t[:, k, :], xt[k, :, c * 512 : (c + 1) * 512]
                    )
                x_tiles[c] = x_t

            # DMA issue order tracks phase-1 consumption order: tiny
            # constants first (a Q-bias/mask arriving behind megabytes
            # of weights stalls the whole phase-2 pipeline), then
            # {x0,wq} pairs per k (phase-1 Q is k-outer), all-K, all-V,
            # chunk 1, and wo in 128 KB chunks so no single transfer
            # hogs a queue.
            # ---- PE pre-warm -------------------------------------
            # No DMA packet lands before ~9 us (engine/queue boot), so
            # the PE would sit idle and start the real stream at the
            # throttled 1.2 GHz HAM state.  A stream of dummy matmuls
            # on a memset scratch tile keeps the PE busy from ~0.5 us,
            # so HAM is at 2.4 GHz (and stays there) when the first
            # projection matmul fires at ~11 us.
            scr_t = constp.tile([128, 512], BF16, tag="scr")
            nc.gpsimd.memset(scr_t[:], 0.0)
            warm_ps = ps_st.tile([128, 512], F32, tag="st",
                                 name="warm_ps")
            for i in range(10):
                nc.tensor.matmul(
                    warm_ps[:], scr_t[:, 0:128], scr_t[:],
                    start=True, stop=True,
                )

            bq_t = constp.tile([128, HPC], F32, tag="bq")
            mask_t = constp.tile([128, 128], F32, tag="mask")
            ones_m = constp.tile([128, 128], BF16, tag="ones_m")
            x1 = xtp.tile([128, KT, 512], BF16, tag="x", name="x1")
            x_tiles[1] = x1
            # One dma_start lands on ONE of the 16 DMA engines, and a
            # [128, W] tile is 128 row-packets processed serially there
            # (~53 ns each): a whole tile takes ~6.8 us to arrive no
            # matter W, a 64-row half ~3.4 us.  Each queue issues one
            # dma_start per ~650 ns, serially.  The phase-1 tensors are
            # issued in EXACT k-outer consumption order, round-robined
            # over the three DMA-capable queues (sync / scalar /
            # gpsimd), halves first, so delivery tracks consumption
            # with a ~3.5 us lead.
            items = []
            for k in range(KT):
                items += [
                    (x0_k[k][0:64, :], xt[k, 0:64, 0:512]),
                    (x0_k[k][64:128, :], xt[k, 64:128, 0:512]),
                    (wq_k[k][0:64, :], wq[k, 0:64, :]),
                    (wq_k[k][64:128, :], wq[k, 64:128, :]),
                ]
            for k in range(KT):
                items += [
                    (wk_k[k][0:64, :], wk[k, 0:64, :]),
                    (wk_k[k][64:128, :], wk[k, 64:128, :]),
                ]
                if k == 2:
                    items.append((bq_t[:], bq2.rearrange("h p -> p h")))
                    items.append((mask_t[:], mblk[:]))
                    items.append((ones_m[:], onem[:]))
            for k in range(KT):
                items += [
                    (wv_k[k][0:64, :], wv[k, 0:64, :]),
                    (wv_k[k][64:128, :], wv[k, 64:128, :]),
                ]
            queues = [nc.sync, nc.scalar, nc.gpsimd]
            for j, (dst, src) in enumerate(items):
                queues[j % 3].dma_start(dst, src)
            for k in range(KT):
                nc.sync.dma_start(x1[:, k, :], xt[k, :, 512:1024])
            for h in range(HPC):
                for j in range(4):
                    nc.sync.dma_start(
                        wo_t[:, h, j * 512 : (j + 1) * 512],
                        wo[h, :, j * 512 : (j + 1) * 512],
                    )
            # warm the scalar engine's exp table before the main stream
            warm_t = constp.tile([128, 1], F32, tag="warm")
            nc.scalar.activation(warm_t[:], mask_t[:, 0:1], AF.Exp, scale=0.0)

            qt = qkvp.tile([128, HPC, S], BF16, tag="qt")
            ktt = qkvp.tile([128, HPC, S], BF16, tag="ktt")
            vt = qkvp.tile([128, NST, GW], BF16, tag="vt")
            a_sbs = {}  # (h, c) -> normalized attnT tile (bf16)

            # ---------------- stream generators -----------------
            def p0_units():
                """Chunk-0 projections, k-outer: 204 yields.

                Q then K then V, each k-outer with 4 PSUM banks (one
                per head / s-tile), so the PE consumes {x0[k], w[k]}
                pairs in DMA arrival order and compute starts as soon
                as the first 256 KB land instead of after the full
                4 MB.
                """
                load_x(2)
                ps_q = [ps_work.tile([128, 512], F32, tag="ps",
                                     name=f"q0_ps{h}")
                        for h in range(HPC)]
                for k in range(KT):
                    for h in range(HPC):
                        nc.tensor.matmul(
                            ps_q[h][:],
                            wq_k[k][:, h * DH : (h + 1) * DH],
                            x0_k[k][:],
                            start=(k == 0),
                            stop=(k == KT - 1),
                        )
                        if k < KT - 1 or h < HPC - 1:
                            yield
                for h in range(HPC):
                    # DVE, not ACT: the scalar queue is busy issuing
                    # prologue DMAs and would stall the PSUM ring
                    nc.vector.tensor_scalar_add(
                        qt[:, h, 0:512], ps_q[h][:], bq_t[:, h : h + 1]
                    )
                    yield
                ps_k = [ps_work.tile([128, 512], F32, tag="ps",
                                     name=f"k0_ps{h}") for h in range(HPC)]
                for k in range(KT):
                    for h in range(HPC):
                        nc.tensor.matmul(
                            ps_k[h][:],
                            wk_k[k][:, h * DH : (h + 1) * DH],
                            x0_k[k][:],
                            start=(k == 0),
                            stop=(k == KT - 1),
                        )
                        if k < KT - 1 or h < HPC - 1:
                            yield
                for h in range(HPC):
                    nc.vector.tensor_copy(ktt[:, h, 0:512], ps_k[h][:])
                    yield
                ps_v = [ps_work.tile([128, GW], F32, tag="ps",
                                     name=f"v0_ps{s}") for s in range(4)]
                for k in range(KT):
                    for s in range(4):
                        nc.tensor.matmul(
                            ps_v[s][:],
                            x0_k[k][:, s * 128 : (s + 1) * 128],
                            wv_k[k][:],
                            start=(k == 0),
                            stop=(k == KT - 1),
                        )
                        if k < KT - 1 or s < 3:
                            yield
                for s in range(4):
                    nc.vector.tensor_copy(vt[:, s, :], ps_v[s][:])
                    yield

            def p_units(c):
                """Projections for query chunk c>=1: 192 yields.

                All-Q, all-K, all-V order matches the prologue's DMA
                issue order so phase 1 never waits on weights; the
                4-deep work-pool ring hides every PSUM->SBUF copy.
                """
                if c + 2 < NQC:
                    load_x(c + 2)
                x_t = x_tiles[c]

                for w_k, dst, is_q in (
                    (wq_k, qt, True),
                    (wk_k, ktt, False),
                ):
                    for h in range(HPC):
                        ps = ps_work.tile([128, 512], F32, tag="ps",
                                          name="qk_ps")
                        hsl = slice(h * DH, (h + 1) * DH)
                        for k in range(KT):
                            nc.tensor.matmul(
                                ps[:],
                                w_k[k][:, hsl],
                                x_t[:, k, :],
                                start=(k == 0),
                                stop=(k == KT - 1),
                            )
                            if k < KT - 1:
                                yield
                        c0 = c * 512
                        if is_q:
                            nc.scalar.add(
                                dst[:, h, c0 : c0 + 512],
                                ps[:],
                                bq_t[:, h : h + 1],
                            )
                        else:
                            nc.vector.tensor_copy(
                                dst[:, h, c0 : c0 + 512], ps[:]
                            )
                        yield
                for s in range(4):
                    ps = ps_work.tile([128, GW], F32, tag="ps",
                                      name="v_ps")
                    for k in range(KT):
                        nc.tensor.matmul(
                            ps[:],
                            x_t[:, k, s * 128 : (s + 1) * 128],
                            wv_k[k][:],
                            start=(k == 0),
                            stop=(k == KT - 1),
                        )
                        if k < KT - 1:
                            yield
                    nc.vector.tensor_copy(vt[:, c * 4 + s, :], ps[:])
                    yield

            def a_units(c):
                """Attention for query chunk c: 4*(4*(c+1)+5) yields.

                Key tiles are processed diagonal-block-first (i = 4c..
                4c+3 then 0..4c-1) so the start=True PV/score tile is
                the full-width diagonal tile at lo=0.  The causal mask
                is preloaded into the score PSUM bank by the DVE and
                the diagonal matmul accumulates onto it, keeping the
                score->exp->PV chain free of cross-engine hops.
                """
                n_kt = 4 * (c + 1)
                order = list(range(4 * c, n_kt)) + list(range(4 * c))
                qsl = slice(c * 512, (c + 1) * 512)
                for h in range(HPC):
                    hsl = slice(h * DH, (h + 1) * DH)
                    attn_ps = ps_at.tile([128, 512], F32, tag="ps",
                                         name="attn_ps")
                    # bf16 accumulator: each lane only sums 16 exps
                    # (<=0.1% den error), adds run at 2x DVE rate, and
                    # the den matmul consumes it with no cast.
                    exp_sum = esp.tile([128, 512], BF16, tag="es",
                                       name="exp_sum")
                    expts = {}
                    los = {}

                    def score(i):
                        st = ps_st.tile([128, 512], F32, tag="st",
                                        name="st")
                        diag = i >= 4 * c
                        lo = 128 * (i - 4 * c) if diag else 0
                        los[i] = lo
                        nc.tensor.matmul(
                            st[:, lo:],
                            ktt[:, h, i * 128 : (i + 1) * 128],
                            qt[:, h, c * 512 + lo : (c + 1) * 512],
                            start=True,
                            stop=True,
                        )
                        if diag:
                            nc.vector.tensor_tensor(
                                st[:, lo : lo + 128],
                                st[:, lo : lo + 128],
                                mask_t[:],
                                op=ALU.add,
                            )
                        expt = expp.tile([128, 512], BF16, tag="exp",
                                         name="expt")
                        nc.scalar.activation(
                            expt[:, lo:], st[:, lo:], AF.Exp, scale=SCALE
                        )
                        expts[i] = expt

                    def pv_acc(j, first, last):
                        # PV matmul, then the exp_sum accumulate for
                        # the SAME tile.  Issued one unit after
                        # score(i) so the DVE's accumulate never sits
                        # ahead of the next tile's mask preload in the
                        # in-order DVE queue.
                        i = order[j]
                        lo = los[i]
                        nc.tensor.matmul(
                            attn_ps[:, lo:],
                            vt[:, i, hsl],
                            expts[i][:, lo:],
                            start=first,
                            stop=last,
                        )
                        if first:
                            nc.vector.tensor_copy(exp_sum[:], expts[i][:])
                        else:
                            nc.vector.tensor_tensor(
                                exp_sum[:, lo:], exp_sum[:, lo:],
                                expts[i][:, lo:], op=ALU.add,
                            )

                    score(order[0])
                    yield
                    for j in range(1, n_kt):
                        score(order[j])
                        pv_acc(j - 1, j == 1, False)
                        yield
                    pv_acc(n_kt - 1, n_kt == 1, True)
                    yield
                    yield  # spacer: filler covers the DVE accum tail
                    yield
                    den_ps = ps_st.tile([128, 512], F32, tag="st",
                                        name="den_ps")
                    nc.tensor.matmul(
                        den_ps[:], ones_m[:], exp_sum[:],
                        start=True, stop=True,
                    )
                    yield
                    yield  # spacer: filler covers the recip/mult chain
                    rc = rcp.tile([128, 512], F32, tag="rc", name="rc")
                    nc.vector.reciprocal_approx_fast(
                        out=rc[:], in_=den_ps[:]
                    )
                    a_sb = asbp.tile([128, 512], BF16, tag="attnT",
                                     name="a_sb")
                    nc.vector.tensor_tensor(
                        a_sb[:], attn_ps[:], rc[:], op=ALU.mult
                    )
                    a_sbs[(h, c)] = a_sb
                    yield

            def op_units(c):
                """Out-projection for query chunk c: 64 yields.

                The four nch tiles of one 128-query row block land in a
                single [128, 2048] SBUF tile and leave as ONE contiguous
                512 KB DMA: per-call issue cost (~650 ns of serial
                DMA_DIRECT2D) is paid 16x per core instead of 128x, so
                the post-compute drain tail collapses.
                """
                for qs in range(4):
                    row0 = c * 512 + qs * 128
                    o_row = ostp.tile([128, D], BF16, tag="ost",
                                      name="o_row")
                    for nch in range(4):
                        ps = ps_work.tile([128, 512], F32, tag="ps",
                                        name="op_ps")
                        for h in range(HPC):
                            nc.tensor.matmul(
                                ps[:],
                                a_sbs[(h, c)][:, qs * 128 : (qs + 1) * 128],
                                wo_t[:, h, nch * 512 : (nch + 1) * 512],
                                start=(h == 0),
                                stop=(h == HPC - 1),
                            )
                            if h < HPC - 1:
                                yield
                        nc.vector.tensor_copy(
                            o_row[:, nch * 512 : nch * 512 + 384],
                            ps[:, :384],
                        )
                        nc.scalar.copy(
                            o_row[:, nch * 512 + 384 : (nch + 1) * 512],
                            ps[:, 384:],
                        )
                        yield
                    nc.sync.dma_start(out[row0 : row0 + 128, :], o_row[:])

            # ---------------- phase schedule -----------------
            NA = lambda c: 4 * (4 * (c + 1) + 6)
            _weave([(p0_units(), 201)])
            _weave([(p_units(1), 192), (a_units(0), NA(0))])
            _weave([(p_units(2), 192), (a_units(1), NA(1)),
                    (op_units(0), 64)])
            _weave([(p_units(3), 192), (a_units(2), NA(2))])
            _weave([(a_units(3), NA(3)), (op_units(1), 64),
                    (op_units(2), 64)])
            _weave([(op_units(3), 64)])
    nc.compile()
    return nc


def _get_nc():
    if "nc" not in _NC_CACHE:
        _NC_CACHE["nc"] = _build()
    return _NC_CACHE["nc"]


def kernel(x, mask, Wq, bq, Wk, bk, Wv, bv, Wo, bo):
    x = np.asarray(x, dtype=np.float32)
    Wq = np.asarray(Wq, dtype=np.float32)
    Wk = np.asarray(Wk, dtype=np.float32)
    Wv = np.asarray(Wv, dtype=np.float32)
    Wo = np.asarray(Wo, dtype=np.float32)
    bq = np.asarray(bq, dtype=np.float32)
    bv = np.asarray(bv, dtype=np.float32)
    bo = np.asarray(bo, dtype=np.float32)

    nc = _get_nc()

    # per-batch transposed x, bf16: (KT, 128, S)
    xts = [
        np.ascontiguousarray(
            x[b].T.reshape(KT, 128, S)
        ).astype(BF)
        for b in range(B)
    ]
    kl = np.arange(128)
    mblk = np.where(kl[:, None] <= kl[None, :], 0.0, NEG).astype(np.float32)
    onem = np.ones((128, 128), dtype=BF)

    in_maps = []
    for c in range(NCORES):
        b, g = c // HPC, c % HPC
        cols = slice(g * GW, (g + 1) * GW)
        in_maps.append(
            {
                "xt": xts[b],
                "wq": np.ascontiguousarray(Wq[:, cols]).reshape(
                    KT, 128, GW
                ).astype(BF),
                "wk": np.ascontiguousarray(Wk[:, cols]).reshape(
                    KT, 128, GW
                ).astype(BF),
                "wv": np.ascontiguousarray(Wv[:, cols]).reshape(
                    KT, 128, GW
                ).astype(BF),
                "wo": np.ascontiguousarray(Wo[cols, :]).reshape(
                    HPC, 128, D
                ).astype(BF),
                "bq2": np.ascontiguousarray(bq[cols]).reshape(HPC, 128),
                "mblk": mblk,
                "onem": onem,
            }
        )

    trace = bool(int(os.environ.get("BASS_ATTN_TRACE", "0")))
    try:
        res = run_bass_kernel_spmd(
            nc, in_maps, core_ids=list(range(NCORES)), trace=trace
        )
    except Exception:
        # transient device errors (e.g. a wedged core from a prior run)
        # usually clear on retry
        res = run_bass_kernel_spmd(
            nc, in_maps, core_ids=list(range(NCORES)), trace=trace
        )
    if trace:
        _NC_CACHE["last_result"] = res

    outs = np.empty((B, S, D), dtype=np.float32)
    for b in range(B):
        acc = res.results[b * HPC]["out"].astype(np.float32)
        for g in range(1, HPC):
            acc += res.results[b * HPC + g]["out"].astype(np.float32)
        outs[b] = acc
    # bv's effect: softmax rows sum to 1, so attn = attn_nobv + bv per head
    # -> out += bv @ Wo (exact). bo added directly. bk cancels in softmax.
    corr = (bv.astype(np.float64) @ Wo.astype(np.float64)) + np.asarray(
        bo, dtype=np.float64
    )
    outs += corr.astype(np.float32)
    return outs

